# revision 58
# baseline (speedup 1.0000x reference)
"""Trainium2 Bass kernel for nn_AttentionSortNet (sparse_attention).

Per bh slice (data-parallel over bh across 8 cores):
  b_q = bucket-mean(q), b_k = bucket-mean(k)          (64 buckets x 128 elems)
  sq = b_q + q_pos, sk = b_k + k_pos
  R  = sq @ sk^T                                       (64 x 64)
  K  = exp((ln(relu(R)+eps) + gumbel) / T)
  8x Sinkhorn; out = normalized K

Design (per core, 4 bh = 2 bh-pairs; ~46.5us vs 58.5us bf16 baseline):
  - q/k staged as fp8 e4m3 with per-bucket error diffusion on the host:
    the quantization residual is carried along the 128 seq positions of
    each bucket, so bucket SUMS of the fp8 values are near-exact
    (4.5e-3 total rel err vs the 2e-2 budget). Halves HBM traffic vs
    bf16 to ~4.2 MiB/core (~13 us stream).
  - bucket means entirely on the PE as fp8 DoubleRow matmuls against a
    duplicated scaled eye ([128, 2, 128] stationary): each instruction
    consumes 1024 elems/partition (2 elem/cycle/partition at full
    clock), accumulating ri-lane partial means in PSUM. The ACT engine
    drains PSUM (Copy, in every table), a contiguous halving tree on
    DVE sums the lanes, pos_emb rides the last add. The last group uses
    ri=4 for a shallower tail tree.
  - the tensor engine needs ~3us of CONTINUOUS execution to reach full
    clock (427->216ns per 512-row mm) and any ~0.4us gap drops it
    back: NWARM dummy matmuls warm it up under the DMA stream, and the
    emission order keeps the PE queue dense.
  - cross-engine waits lower to monotone per-engine counters (consumer
    waits "producer engine completed >= N", N = producer-queue position
    at emission). Consumers are emitted immediately after their
    producers; unrelated fill work comes after, never between.
  - chunk completion follows descriptor-generation order, but rows
    below ~2KB make descriptors that clog single DMA engines (the
    stream tail dribbles): 4KB rows, with 2KB rows only for the last
    group's tail chunks.
  - Sinkhorn in scale-vector form: E = diag(a) K diag(b) with
    a <- 1/(K b), b <- 1/(K^T a) -- algebraically identical to the
    reference iteration. Each half-step is ONE 4-column matvec (bf16
    stationary K or K^T, masked bf16 moving columns per (v,h') bh)
    plus two [64,2] DVE reciprocals (~0.65us/half-step), instead of
    rescaling the full 128x128 matrix. The mask layout makes unused
    moving columns stay zero across iterations (memset once).
  - final assembly E = (K*a_bcast) * b_replicated: a is applied per
    partition block during the last matvec's window; b is replicated
    across partitions with one small selector matmul (selv4 [4,128],
    the h'-mask collapses the selector sum), one output DMA on the
    idle gpsimd SWDGE queue (HWDGE pays ~1.4us issue latency there).
  - one ACT table set (natural_log_exp_and_others) force-loaded up
    front via a hand-inserted InstLoadActFuncSet (no per-activation
    table-switch thrash for eg/Ln/Exp).
"""
import sys

sys.path.insert(0, "/opt/trn_rl_repo")

import numpy as np
import ml_dtypes

import concourse.bass as bass
import concourse.bacc as bacc
import concourse.mybir as mybir
from concourse import tile
from concourse.bass_utils import run_bass_kernel_spmd
from concourse.dve_ops import TENSOR_TENSOR_REDUCE

HEADS = 8
BUCKETS = 64
DIM = 64
TEMP = 0.7
EPS = 1e-6
N_CORES = 8
BH = 32
SEQ = 8192
NBH = BH // N_CORES        # 4 bh per core
PAIRS = NBH // 2           # 2 bh-pairs per core
SINKHORN_ITER = 8

F32 = mybir.dt.float32
BF16 = mybir.dt.bfloat16
FP8 = mybir.dt.float8e4
AF = mybir.ActivationFunctionType
AX = mybir.AxisListType
ALU = mybir.AluOpType
DR = mybir.MatmulPerfMode.DoubleRow

# per (pair, tensor): seq elems per partition = 8192 (fp8 = 8KB rows).
# Uniform 4KB rows: small-row chunks generate tiny DMA descriptors that
# pile up on one engine and make the stream tail dribble for ~3us.
CHUNK_SZ = {
    (0, 0): (4096, 4096),
    (0, 1): (4096, 4096),
    (1, 0): (4096, 4096),
    (1, 1): (4096, 2048, 2048),
}

# ACT Reciprocal is rejected by bass (known accuracy issues), so both
# per-half-step reciprocals run sequentially on DVE.
ACT_RECIP = False


def _build_program():
    nc = bacc.Bacc("TRN2", target_bir_lowering=False, debug=False, num_devices=N_CORES)

    q_d = nc.dram_tensor("q", [NBH, SEQ, DIM], FP8, kind="ExternalInput")
    k_d = nc.dram_tensor("k", [NBH, SEQ, DIM], FP8, kind="ExternalInput")
    # pre-stacked on host: [128=(v,row), pair, 64]
    qp_d = nc.dram_tensor("posq", [128, PAIRS, DIM], F32, kind="ExternalInput")
    kp_d = nc.dram_tensor("posk", [128, PAIRS, DIM], F32, kind="ExternalInput")
    g_d = nc.dram_tensor("gum", [128, PAIRS, BUCKETS], F32, kind="ExternalInput")
    eyeb2_d = nc.dram_tensor("eyeb2", [128, 256], FP8, kind="ExternalInput")
    eye_d = nc.dram_tensor("eye", [128, 128], F32, kind="ExternalInput")
    eyeh_d = nc.dram_tensor("eyeh", [128, 128], BF16, kind="ExternalInput")
    sel_d = nc.dram_tensor("selv4", [4, 128], BF16, kind="ExternalInput")
    out_d = nc.dram_tensor("out", [NBH, BUCKETS, BUCKETS], F32, kind="ExternalOutput")

    with tile.TileContext(nc) as tc:
        with (
            tc.tile_pool(name="const", bufs=1) as constp,
            tc.tile_pool(name="data", bufs=13) as datap,
            tc.tile_pool(name="work", bufs=3) as workp,
            tc.tile_pool(name="small", bufs=4) as smallp,
            tc.tile_pool(name="persist", bufs=1) as persistp,
            tc.tile_pool(name="pacc", bufs=2, space=bass.MemorySpace.PSUM) as pacc,
            tc.tile_pool(name="ptr", bufs=2, space=bass.MemorySpace.PSUM) as ptr,
            tc.tile_pool(name="pR", bufs=1, space=bass.MemorySpace.PSUM) as pR,
            tc.tile_pool(name="pmv", bufs=1, space=bass.MemorySpace.PSUM) as pmv,
            tc.tile_pool(name="pscr", bufs=1, space=bass.MemorySpace.PSUM) as pscr,
            tc.tile_pool(name="pbrep", bufs=1, space=bass.MemorySpace.PSUM) as pbrep,
        ):
            # small consts FIRST on the sync HWDGE queue: their descriptors
            # must hit the DMA engines before the ~4 MiB of q/k descriptors,
            # or eyeb2/gum only land after the whole stream drains.
            eyeb2 = constp.tile([128, 256], FP8, tag="eyeb2")
            nc.sync.dma_start(eyeb2[:], eyeb2_d[:])
            gum = constp.tile([128, PAIRS, BUCKETS], F32, tag="gum")
            nc.sync.dma_start(gum[:], g_d[:])
            posq = constp.tile([128, PAIRS, DIM], F32, tag="posq")
            nc.scalar.dma_start(posq[:], qp_d[:])
            posk = constp.tile([128, PAIRS, DIM], F32, tag="posk")
            nc.scalar.dma_start(posk[:], kp_d[:])
            eye = constp.tile([128, 128], F32, tag="eye")
            nc.scalar.dma_start(eye[:], eye_d[:])
            eyeh = constp.tile([128, 128], BF16, tag="eyeh")
            nc.scalar.dma_start(eyeh[:], eyeh_d[:])
            selv4 = constp.tile([4, 128], BF16, tag="selv4")
            nc.scalar.dma_start(selv4[:], sel_d[:])

            # big q/k chunks ride the gpsimd SWDGE queue
            chunk_map = {}
            for pi in range(PAIRS):
                for ti, src in ((0, q_d), (1, k_d)):
                    view = src[2 * pi : 2 * pi + 2].rearrange(
                        "b (bu sl) d -> (b bu) (sl d)", bu=BUCKETS, sl=SEQ // BUCKETS
                    )
                    off = 0
                    lst = []
                    eng = nc.sync if (pi == 1 and ti == 1) else nc.gpsimd
                    for csz in CHUNK_SZ[(pi, ti)]:
                        ch = datap.tile([128, csz], FP8, tag="data")
                        eng.dma_start(ch[:], view[:, off : off + csz])
                        lst.append((ch, off, csz))
                        off += csz
                    chunk_map[(pi, ti)] = lst

            # eg = exp(g/T) during the DMA window (ACT is otherwise idle)
            eg = constp.tile([128, PAIRS, BUCKETS], F32, tag="eg")
            nc.scalar.activation(eg[:], gum[:], AF.Exp, scale=1.0 / TEMP)
            # Ln table warm during the DMA window
            tw = constp.tile([128, 1], F32, tag="tw")
            nc.vector.memset(tw[:], 1.0)
            nc.scalar.activation(tw[:], tw[:], AF.Ln)

            # Sinkhorn state: masked moving tiles (bf16: the matvec operands
            # and stationaries are bf16, validated 5.9e-3 total rel err).
            # Columns are (h',v') with c = 2h'+v'. Unwritten positions must
            # stay zero -> memset once.
            M1 = persistp.tile([128, 4], BF16, tag="M1")   # b side, part (h,j)
            M2 = persistp.tile([128, 4], BF16, tag="M2")   # a side, part (v,i)
            nc.vector.memset(M1[:], 0.0)
            nc.vector.memset(M2[:], 0.0)
            seed = persistp.tile([128, PAIRS], F32, tag="seed")
            E0b = persistp.tile([128, 2 * BUCKETS], BF16, tag="E0b")  # K (bf16 everywhere)
            KTb = persistp.tile([128, 2 * BUCKETS], BF16, tag="KTb")  # K^T bf16 stationary
            T1 = persistp.tile([128, 2 * BUCKETS], F32, tag="T1")    # K*a

            eyeb2v = eyeb2[:].rearrange("p (two m) -> p two m", two=2)
            sT = {}

            # PE p-state warmup: the tensor engine needs ~3us of continuous
            # execution to reach full clock (427ns -> 216ns per 512-row mm).
            # Run dummy DoubleRow matmuls on scratch data while the first
            # chunk streams in, so the real matmuls start at full speed.
            wsrc = constp.tile([128, 1024], FP8, tag="wsrc")
            nc.vector.memset(wsrc[:], 0.0)
            wacc = pacc.tile([128, 512], F32, tag="acc")
            NWARM = 8
            for i in range(NWARM):
                nc.tensor.matmul(
                    wacc[:],
                    eyeb2v,
                    wsrc[:].rearrange("p (two f) -> p two f", two=2),
                    start=(i == 0),
                    stop=(i == NWARM - 1),
                    perf_mode=DR,
                )

            def emit_mms(pi, ti, step=512):
                # DoubleRow eye-pair matmuls: each consumes 2*step elems per
                # partition, accumulating step "ri-lane" partial means.
                acc = pacc.tile([128, step], F32, tag="acc")
                total = SEQ // (2 * step)
                m = 0
                for ch, coff, csz in chunk_map[(pi, ti)]:
                    for l in range(0, csz, 2 * step):
                        nc.tensor.matmul(
                            acc[:],
                            eyeb2v,
                            ch[:, l : l + 2 * step].rearrange(
                                "p (two f) -> p two f", two=2
                            ),
                            start=(m == 0),
                            stop=(m == total - 1),
                            perf_mode=DR,
                        )
                        m += 1
                return acc, step

            def emit_drain_tree(accs, pi, pos):
                # drain PSUM via the ACT engine (close to PSUM; Copy is in
                # every table), then a contiguous halving tree over the ri
                # lanes on DVE (dual-PSUM reads are not allowed), with the
                # pos_emb add as the last level.
                acc, step = accs
                c0 = workp.tile([128, step], F32, tag=f"c0{step}")
                nc.scalar.activation(c0[:], acc[:], AF.Copy)
                cur = c0
                w = step
                while w > 64:
                    nxt = workp.tile([128, w // 2], F32, tag=f"t{w}")
                    nc.vector.tensor_tensor(
                        out=nxt[:], in0=cur[:, 0 : w // 2], in1=cur[:, w // 2 : w],
                        op=ALU.add,
                    )
                    cur, w = nxt, w // 2
                s2 = workp.tile([128, DIM], F32, tag="s2")
                nc.vector.tensor_tensor(
                    out=s2[:], in0=cur[:], in1=pos[:, pi, :], op=ALU.add
                )
                return s2

            def emit_tp(pi, nm, s2):
                tps = ptr.tile([64, 128], F32, tag="tp")
                nc.tensor.matmul(
                    tps[:], s2[:], eye[:], is_transpose=True, start=True, stop=True,
                )
                s_t = persistp.tile([64, 128], F32, tag=f"sT{nm}{pi}")
                nc.vector.tensor_copy(s_t[:], tps[:])
                sT[(nm, pi)] = s_t

            def emit_R(pi):
                Rps = pR.tile([128, BUCKETS], F32, tag="R")
                for v in range(2):
                    nc.tensor.matmul(
                        Rps[64 * v : 64 * (v + 1), :],
                        sT[("q", pi)][:, 64 * v : 64 * (v + 1)],
                        sT[("k", pi)][:, 64 * v : 64 * (v + 1)],
                        start=True,
                        stop=True,
                    )
                return Rps

            def emit_y(pi, Rps):
                y = workp.tile([128, BUCKETS], F32, tag="y")
                nc.vector.tensor_scalar(
                    out=y[:], in0=Rps[:], scalar1=0.0, scalar2=EPS,
                    op0=ALU.max, op1=ALU.add,
                )
                return y

            def emit_strip(pi, y):
                # K column strip h=pi: exp((ln(relu R + eps))/T) * exp(g/T),
                # with row sums accumulated as the Sinkhorn seed.
                u = workp.tile([128, BUCKETS], F32, tag="u")
                nc.scalar.activation(u[:], y[:], AF.Ln)
                vv = workp.tile([128, BUCKETS], F32, tag="vv")
                nc.scalar.activation(vv[:], u[:], AF.Exp, scale=1.0 / TEMP)
                nc.vector._custom_dve(
                    TENSOR_TENSOR_REDUCE,
                    out=E0b[:, 64 * pi : 64 * (pi + 1)],
                    in0=vv[:],
                    in1=eg[:, pi, :],
                    s0=0.0,
                    s1=1.0,
                    accum_out=seed[:, pi : pi + 1],
                )

            def emit_striptp(pi):
                tstr = pscr.tile([64, 128], BF16, tag="scr")
                nc.tensor.matmul(
                    tstr[:], E0b[:, 64 * pi : 64 * (pi + 1)], eyeh[:],
                    is_transpose=True, start=True, stop=True,
                )
                nc.vector.tensor_copy(KTb[64 * pi : 64 * (pi + 1), :], tstr[:])

            # Emission ordered by data-readiness so no engine queue blocks on
            # a dependency that is satisfied later than its successors' data:
            # chunks complete in issue order (p0q, p0k, p1q, p1k).
            # Cross-engine waits lower to monotone per-engine counters: a
            # consumer waits for "producer engine completed >= N" where N is
            # the producer-queue position at emission time. So every consumer
            # must be emitted IMMEDIATELY after its producer's last
            # instruction on that engine — anything emitted in between
            # becomes a false dependency. Order below is tuned under that
            # rule with chunks completing in issue order.
            acc00 = emit_mms(0, 0)
            s2_00 = emit_drain_tree(acc00, 0, posq)
            acc01 = emit_mms(0, 1)
            s2_01 = emit_drain_tree(acc01, 0, posk)
            emit_tp(0, "q", s2_00)
            acc10 = emit_mms(1, 0)
            emit_tp(0, "k", s2_01)
            s2_10 = emit_drain_tree(acc10, 1, posq)
            # last group with ri=4: a shallower drain tree on the critical tail
            acc11 = emit_mms(1, 1, step=256)
            s2_11 = emit_drain_tree(acc11, 1, posk)
            R0 = emit_R(0)
            emit_tp(1, "q", s2_10)
            y0 = emit_y(0, R0)
            emit_tp(1, "k", s2_11)
            R1 = emit_R(1)
            y1 = emit_y(1, R1)
            emit_strip(0, y0)
            emit_strip(1, y1)

            # ---- Sinkhorn, scale-vector form ----
            # bf16 scale vectors + stationaries: total rel err validated at
            # 5.9e-3 on the host against the 2e-2 budget.
            lp = nc.allow_low_precision(reason="bf16 sinkhorn scale vectors")
            lp.__enter__()
            # a_1 = 1/rowsums: write into M2 block v at cols {v, 2+v}
            for v in range(2):
                sl = slice(64 * v, 64 * (v + 1))
                dst = M2[sl].rearrange("p (h w) -> p h w", h=2)[:, :, v]
                nc.vector.reciprocal(dst, seed[sl, :])
            # KT rows are only needed by the second matvec; emitting them
            # after the seed reciprocals keeps the first matvec unblocked.
            emit_striptp(0)
            emit_striptp(1)

            for t in range(SINKHORN_ITER):
                # b-update: b_raw[(h,j), (h',v')] = sum_i K[(v',i),(h,j)] a[(v',i),h']
                b_raw = pmv.tile([128, 4], F32, tag="mv")
                nc.tensor.matmul(b_raw[:], E0b[:], M2[:], start=True, stop=True)
                # valid cols for block h: {2h, 2h+1} (contiguous)
                nc.vector.reciprocal(M1[0:64, 0:2], b_raw[0:64, 0:2])
                if ACT_RECIP and t > 0:
                    nc.scalar.activation(
                        M1[64:128, 2:4], b_raw[64:128, 2:4], AF.Reciprocal
                    )
                else:
                    nc.vector.reciprocal(M1[64:128, 2:4], b_raw[64:128, 2:4])
                if t == SINKHORN_ITER - 1:
                    break
                # a-update: a_raw[(v,i), (h',v')] = sum_j K[(v,i),(h',j)] b[(h',j),v']
                a_raw = pmv.tile([128, 4], F32, tag="mv")
                nc.tensor.matmul(a_raw[:], KTb[:], M1[:], start=True, stop=True)
                # valid cols for block v: {v, 2+v} (stride 2)
                for v in range(2):
                    sl = slice(64 * v, 64 * (v + 1))
                    src = a_raw[sl].rearrange("p (h w) -> p h w", h=2)[:, :, v]
                    dst = M2[sl].rearrange("p (h w) -> p h w", h=2)[:, :, v]
                    if ACT_RECIP and v == 1 and t > 0:
                        nc.scalar.activation(dst, src, AF.Reciprocal)
                    else:
                        nc.vector.reciprocal(dst, src)
                if t == SINKHORN_ITER - 2:
                    # M2 now holds the final a; fold it into K while the last
                    # b half-step runs: T1[(v,i),(h,j)] = K * a[(v,i),h]
                    for v in range(2):
                        sl = slice(64 * v, 64 * (v + 1))
                        av = M2[sl].rearrange("p (h w) -> p h w", h=2)[:, :, v]
                        nc.vector.tensor_tensor(
                            out=T1[sl].rearrange("p (h j) -> p h j", h=2),
                            in0=E0b[sl].rearrange("p (h j) -> p h j", h=2),
                            in1=av.unsqueeze(-1).broadcast_to((64, 2, BUCKETS)),
                            op=ALU.mult,
                        )

            # ---- assembly: E = T1 * b_replicated ----
            tpb = pscr.tile([64, 128], BF16, tag="scr")
            nc.tensor.matmul(tpb[0:4, :], M1[:], eyeh[:], is_transpose=True, start=True, stop=True)
            bT = smallp.tile([4, 128], BF16, tag="bT")
            nc.vector.tensor_copy(bT[:], tpb[0:4, :])
            # brep[p=(v,i), (h,j)] = b[(h,j), v]: the h'-mask in M1 collapses
            # the selector sum to exactly the matching b value.
            brep = pbrep.tile([128, 2 * BUCKETS], F32, tag="brep")
            nc.tensor.matmul(brep[:], selv4[:], bT[:], start=True, stop=True)
            Efin = persistp.tile([128, 2 * BUCKETS], F32, tag="Efin")
            for h in range(2):
                sl = slice(64 * h, 64 * (h + 1))
                nc.vector.tensor_tensor(
                    out=Efin[:, sl], in0=T1[:, sl], in1=brep[:, sl], op=ALU.mult
                )
            # single output DMA (one SWDGE descriptor-gen pass on gpsimd,
            # which sits idle here; HWDGE pays ~1.4us of issue latency)
            ov = out_d[:].rearrange("(h v) i j -> (v i) h j", h=2, v=2)
            nc.gpsimd.dma_start(ov, Efin[:].rearrange("p (h j) -> p h j", h=2))
            lp.__exit__(None, None, None)

    _preload_act_table(nc)
    nc.compile()
    return nc


# act_info.json act_func_sets index of natural_log_exp_and_others, the one
# table that serves Ln AND Exp (and Relu/Copy). Pre-loading it up front
# makes Bacc's membership-based fixpoint skip every per-activation
# ACT_TABLE_LOAD (1.28us each) for the Exp/Ln phase.
ACT_SET_LN_EXP = 6


def _preload_act_table(nc, set_id=ACT_SET_LN_EXP):
    load = mybir.InstLoadActFuncSet(
        name=nc.get_next_instruction_name(), act_func_set_id=set_id, ins=[], outs=[]
    )
    for blk in nc.main_func.blocks:
        for idx, inst in enumerate(blk.instructions):
            eng = getattr(inst, "engine", None)
            if eng != mybir.EngineType.Activation:
                continue
            if isinstance(
                inst,
                (
                    mybir.InstActivation,
                    mybir.InstLoadActFuncSet,
                ),
            ):
                load.engine = eng
                nc.register_instruction(load)
                blk.instructions.insert(idx, load)
                return
    raise AssertionError("no activation-engine body instruction found")


_NC = None


def _get_program():
    global _NC
    if _NC is None:
        _NC = _build_program()
    return _NC


def _stack_pairs(a):
    # [4, X, T] (bh-major) -> [128=(v,X), pair, T] with bh = 2*pair + v
    x, t = a.shape[1], a.shape[2]
    return np.ascontiguousarray(
        a.reshape(PAIRS, 2, x, t).transpose(1, 2, 0, 3).reshape(2 * x, PAIRS, t)
    )


def _err_diffuse_fp8(x):
    # x: (BH, SEQ, DIM) f32 -> fp8 e4m3 with the quantization residual
    # carried along the 128 seq positions of each bucket, so bucket sums
    # of the quantized values telescope to near-exact.
    nb, _, d = x.shape
    sl = SEQ // BUCKETS
    xb = x.reshape(nb, BUCKETS, sl, d)
    out = np.empty((nb, BUCKETS, sl, d), dtype=ml_dtypes.float8_e4m3fn)
    carry = np.zeros((nb, BUCKETS, d), np.float32)
    for s in range(sl):
        v = xb[:, :, s, :] + carry
        qv = v.astype(ml_dtypes.float8_e4m3fn)
        out[:, :, s, :] = qv
        carry = v - qv.astype(np.float32)
    return out.reshape(nb, SEQ, d)


def _make_in_maps(inputs):
    q8 = _err_diffuse_fp8(np.asarray(inputs["q"], dtype=np.float32))
    k8 = _err_diffuse_fp8(np.asarray(inputs["k"], dtype=np.float32))
    qpe = np.asarray(inputs["q_pos_emb"], dtype=np.float32)
    kpe = np.asarray(inputs["k_pos_emb"], dtype=np.float32)
    g = np.ascontiguousarray(inputs["gumbel"], dtype=np.float32)

    b = BH // HEADS
    qpos = np.broadcast_to(qpe, (b, HEADS, BUCKETS, DIM)).reshape(BH, BUCKETS, DIM)
    kpos = np.broadcast_to(kpe, (b, HEADS, BUCKETS, DIM)).reshape(BH, BUCKETS, DIM)
    eye = np.eye(128, dtype=np.float32)
    eyeh = np.eye(128, dtype=np.float32).astype(ml_dtypes.bfloat16)
    ey = (np.eye(128, dtype=np.float32) / 128.0).astype(ml_dtypes.float8_e4m3fn)
    eyeb2 = np.ascontiguousarray(np.concatenate([ey, ey], axis=1))
    # selv4[c=(2h'+v'), p] = 1 if v' == p//64
    selv4 = np.zeros((4, 128), np.float32)
    for hp in range(2):
        for vp in range(2):
            selv4[2 * hp + vp, 64 * vp : 64 * vp + 64] = 1.0

    in_maps = []
    for c in range(N_CORES):
        sl = slice(NBH * c, NBH * (c + 1))
        in_maps.append(
            {
                "q": np.ascontiguousarray(q8[sl]),
                "k": np.ascontiguousarray(k8[sl]),
                "posq": _stack_pairs(qpos[sl]),
                "posk": _stack_pairs(kpos[sl]),
                "gum": _stack_pairs(g[sl]),
                "eyeb2": eyeb2,
                "eye": eye,
                "eyeh": eyeh,
                "selv4": selv4.astype(ml_dtypes.bfloat16),
            }
        )
    return in_maps


def run(inputs, trace=False):
    nc = _get_program()
    in_maps = _make_in_maps(inputs)
    res = run_bass_kernel_spmd(
        nc, in_maps, core_ids=list(range(N_CORES)), trace=trace
    )
    out = np.concatenate(
        [res.results[c]["out"] for c in range(N_CORES)], axis=0
    ).astype(np.float32)
    return out, res


def kernel(**inputs) -> np.ndarray:
    out, _ = run(inputs, trace=False)
    return out


# revision 59
# speedup vs baseline: 1.0723x; 1.0723x over previous
"""Trainium2 Bass kernel for nn_AttentionSortNet (sparse_attention).

Per bh slice (data-parallel over bh across 8 cores):
  b_q = bucket-mean(q), b_k = bucket-mean(k)          (64 buckets x 128 elems)
  sq = b_q + q_pos, sk = b_k + k_pos
  R  = sq @ sk^T                                       (64 x 64)
  K  = exp((ln(relu(R)+eps) + gumbel) / T)
  8x Sinkhorn; out = normalized K

Design (per core, 4 bh = 2 bh-pairs; ~46.5us vs 58.5us bf16 baseline):
  - q/k staged as fp8 e4m3 with per-bucket error diffusion on the host:
    the quantization residual is carried along the 128 seq positions of
    each bucket, so bucket SUMS of the fp8 values are near-exact
    (4.5e-3 total rel err vs the 2e-2 budget). Halves HBM traffic vs
    bf16 to ~4.2 MiB/core (~13 us stream).
  - bucket means entirely on the PE as fp8 DoubleRow matmuls against a
    duplicated scaled eye ([128, 2, 128] stationary): each instruction
    consumes 1024 elems/partition (2 elem/cycle/partition at full
    clock), accumulating ri-lane partial means in PSUM. The ACT engine
    drains PSUM (Copy, in every table), a contiguous halving tree on
    DVE sums the lanes, pos_emb rides the last add. The last group uses
    ri=4 for a shallower tail tree.
  - the tensor engine needs ~3us of CONTINUOUS execution to reach full
    clock (427->216ns per 512-row mm) and any ~0.4us gap drops it
    back: NWARM dummy matmuls warm it up under the DMA stream, and the
    emission order keeps the PE queue dense.
  - cross-engine waits lower to monotone per-engine counters (consumer
    waits "producer engine completed >= N", N = producer-queue position
    at emission). Consumers are emitted immediately after their
    producers; unrelated fill work comes after, never between.
  - chunk completion follows descriptor-generation order, but rows
    below ~2KB make descriptors that clog single DMA engines (the
    stream tail dribbles): 4KB rows, with 2KB rows only for the last
    group's tail chunks.
  - Sinkhorn in scale-vector form: E = diag(a) K diag(b) with
    a <- 1/(K b), b <- 1/(K^T a) -- algebraically identical to the
    reference iteration. Each half-step is ONE 4-column matvec (bf16
    stationary K or K^T, masked bf16 moving columns per (v,h') bh)
    plus two [64,2] DVE reciprocals (~0.65us/half-step), instead of
    rescaling the full 128x128 matrix. The mask layout makes unused
    moving columns stay zero across iterations (memset once).
  - final assembly E = (K*a_bcast) * b_replicated: a is applied per
    partition block during the last matvec's window; b is replicated
    across partitions with one small selector matmul (selv4 [4,128],
    the h'-mask collapses the selector sum), one output DMA on the
    idle gpsimd SWDGE queue (HWDGE pays ~1.4us issue latency there).
  - one ACT table set (natural_log_exp_and_others) force-loaded up
    front via a hand-inserted InstLoadActFuncSet (no per-activation
    table-switch thrash for eg/Ln/Exp).
"""
import sys

sys.path.insert(0, "/opt/trn_rl_repo")

import numpy as np
import ml_dtypes

import concourse.bass as bass
import concourse.bacc as bacc
import concourse.mybir as mybir
from concourse import tile
from concourse.bass_utils import run_bass_kernel_spmd
from concourse.dve_ops import TENSOR_TENSOR_REDUCE

HEADS = 8
BUCKETS = 64
DIM = 64
TEMP = 0.7
EPS = 1e-6
N_CORES = 8
BH = 32
SEQ = 8192
NBH = BH // N_CORES        # 4 bh per core
PAIRS = NBH // 2           # 2 bh-pairs per core
SINKHORN_ITER = 8

F32 = mybir.dt.float32
BF16 = mybir.dt.bfloat16
FP8 = mybir.dt.float8e4
AF = mybir.ActivationFunctionType
AX = mybir.AxisListType
ALU = mybir.AluOpType
DR = mybir.MatmulPerfMode.DoubleRow

# per (pair, tensor): seq elems per partition = 8192 (fp8 = 8KB rows).
# Uniform 4KB rows: small-row chunks generate tiny DMA descriptors that
# pile up on one engine and make the stream tail dribble for ~3us.
CHUNK_SZ = {
    (0, 0): (4096, 4096),
    (0, 1): (4096, 4096),
    (1, 0): (4096, 4096),
    (1, 1): (4096, 2048, 2048),
}

# ACT Reciprocal is rejected by bass (known accuracy issues), so both
# per-half-step reciprocals run sequentially on DVE.
ACT_RECIP = False


def _build_program():
    nc = bacc.Bacc("TRN2", target_bir_lowering=False, debug=False, num_devices=N_CORES)

    q_d = nc.dram_tensor("q", [NBH, SEQ, DIM], FP8, kind="ExternalInput")
    k_d = nc.dram_tensor("k", [NBH, SEQ, DIM], FP8, kind="ExternalInput")
    # pre-stacked on host: [128=(v,row), pair, 64]
    qp_d = nc.dram_tensor("posq", [128, PAIRS, DIM], F32, kind="ExternalInput")
    kp_d = nc.dram_tensor("posk", [128, PAIRS, DIM], F32, kind="ExternalInput")
    g_d = nc.dram_tensor("gum", [128, PAIRS, BUCKETS], F32, kind="ExternalInput")
    eyeb2_d = nc.dram_tensor("eyeb2", [128, 256], FP8, kind="ExternalInput")
    eye_d = nc.dram_tensor("eye", [128, 128], F32, kind="ExternalInput")
    eyeh_d = nc.dram_tensor("eyeh", [128, 128], BF16, kind="ExternalInput")
    sel_d = nc.dram_tensor("selv4", [4, 128], BF16, kind="ExternalInput")
    out_d = nc.dram_tensor("out", [NBH, BUCKETS, BUCKETS], F32, kind="ExternalOutput")

    with tile.TileContext(nc) as tc:
        with (
            tc.tile_pool(name="const", bufs=1) as constp,
            tc.tile_pool(name="data", bufs=13) as datap,
            tc.tile_pool(name="work", bufs=3) as workp,
            tc.tile_pool(name="small", bufs=4) as smallp,
            tc.tile_pool(name="persist", bufs=1) as persistp,
            tc.tile_pool(name="pacc", bufs=2, space=bass.MemorySpace.PSUM) as pacc,
            tc.tile_pool(name="ptr", bufs=2, space=bass.MemorySpace.PSUM) as ptr,
            tc.tile_pool(name="pR", bufs=1, space=bass.MemorySpace.PSUM) as pR,
            tc.tile_pool(name="pmv", bufs=1, space=bass.MemorySpace.PSUM) as pmv,
            tc.tile_pool(name="pscr", bufs=1, space=bass.MemorySpace.PSUM) as pscr,
            tc.tile_pool(name="pbrep", bufs=1, space=bass.MemorySpace.PSUM) as pbrep,
        ):
            # small consts FIRST on the sync HWDGE queue: their descriptors
            # must hit the DMA engines before the ~4 MiB of q/k descriptors,
            # or eyeb2/gum only land after the whole stream drains.
            eyeb2 = constp.tile([128, 256], FP8, tag="eyeb2")
            nc.sync.dma_start(eyeb2[:], eyeb2_d[:])
            gum = constp.tile([128, PAIRS, BUCKETS], F32, tag="gum")
            nc.sync.dma_start(gum[:], g_d[:])
            posq = constp.tile([128, PAIRS, DIM], F32, tag="posq")
            nc.sync.dma_start(posq[:], qp_d[:])
            posk = constp.tile([128, PAIRS, DIM], F32, tag="posk")
            nc.sync.dma_start(posk[:], kp_d[:])
            eye = constp.tile([128, 128], F32, tag="eye")
            nc.sync.dma_start(eye[:], eye_d[:])
            eyeh = constp.tile([128, 128], BF16, tag="eyeh")
            nc.sync.dma_start(eyeh[:], eyeh_d[:])
            selv4 = constp.tile([4, 128], BF16, tag="selv4")
            nc.sync.dma_start(selv4[:], sel_d[:])

            # big q/k chunks ride the gpsimd SWDGE queue
            chunk_map = {}
            for pi in range(PAIRS):
                for ti, src in ((0, q_d), (1, k_d)):
                    view = src[2 * pi : 2 * pi + 2].rearrange(
                        "b (bu sl) d -> (b bu) (sl d)", bu=BUCKETS, sl=SEQ // BUCKETS
                    )
                    off = 0
                    lst = []
                    for csz in CHUNK_SZ[(pi, ti)]:
                        ch = datap.tile([128, csz], FP8, tag="data")
                        nc.gpsimd.dma_start(ch[:], view[:, off : off + csz])
                        lst.append((ch, off, csz))
                        off += csz
                    chunk_map[(pi, ti)] = lst

            # eg = exp(g/T) during the DMA window (ACT is otherwise idle)
            eg = constp.tile([128, PAIRS, BUCKETS], F32, tag="eg")
            nc.scalar.activation(eg[:], gum[:], AF.Exp, scale=1.0 / TEMP)
            # Ln table warm during the DMA window
            tw = constp.tile([128, 1], F32, tag="tw")
            nc.vector.memset(tw[:], 1.0)
            nc.scalar.activation(tw[:], tw[:], AF.Ln)

            # Sinkhorn state: masked moving tiles (bf16: the matvec operands
            # and stationaries are bf16, validated 5.9e-3 total rel err).
            # Columns are (h',v') with c = 2h'+v'. Unwritten positions must
            # stay zero -> memset once.
            M1 = persistp.tile([128, 4], BF16, tag="M1")   # b side, part (h,j)
            M2 = persistp.tile([128, 4], BF16, tag="M2")   # a side, part (v,i)
            nc.vector.memset(M1[:], 0.0)
            nc.vector.memset(M2[:], 0.0)
            seed = persistp.tile([128, PAIRS], F32, tag="seed")
            E0b = persistp.tile([128, 2 * BUCKETS], BF16, tag="E0b")  # K (bf16 everywhere)
            KTb = persistp.tile([128, 2 * BUCKETS], BF16, tag="KTb")  # K^T bf16 stationary
            T1 = persistp.tile([128, 2 * BUCKETS], F32, tag="T1")    # K*a

            eyeb2v = eyeb2[:].rearrange("p (two m) -> p two m", two=2)
            sT = {}

            # PE p-state warmup: the tensor engine needs ~3us of continuous
            # execution to reach full clock (427ns -> 216ns per 512-row mm).
            # Run dummy DoubleRow matmuls on scratch data while the first
            # chunk streams in, so the real matmuls start at full speed.
            wsrc = constp.tile([128, 1024], FP8, tag="wsrc")
            nc.vector.memset(wsrc[:], 0.0)
            wacc = pacc.tile([128, 512], F32, tag="acc")
            NWARM = 8
            for i in range(NWARM):
                nc.tensor.matmul(
                    wacc[:],
                    eyeb2v,
                    wsrc[:].rearrange("p (two f) -> p two f", two=2),
                    start=(i == 0),
                    stop=(i == NWARM - 1),
                    perf_mode=DR,
                )

            def emit_mms(pi, ti, step=512):
                # DoubleRow eye-pair matmuls: each consumes 2*step elems per
                # partition, accumulating step "ri-lane" partial means.
                acc = pacc.tile([128, step], F32, tag="acc")
                total = SEQ // (2 * step)
                m = 0
                for ch, coff, csz in chunk_map[(pi, ti)]:
                    for l in range(0, csz, 2 * step):
                        nc.tensor.matmul(
                            acc[:],
                            eyeb2v,
                            ch[:, l : l + 2 * step].rearrange(
                                "p (two f) -> p two f", two=2
                            ),
                            start=(m == 0),
                            stop=(m == total - 1),
                            perf_mode=DR,
                        )
                        m += 1
                return acc, step

            def emit_drain_tree(accs, pi, pos):
                # drain PSUM via the ACT engine (close to PSUM; Copy is in
                # every table), then a contiguous halving tree over the ri
                # lanes on DVE (dual-PSUM reads are not allowed), with the
                # pos_emb add as the last level.
                acc, step = accs
                c0 = workp.tile([128, step], F32, tag=f"c0{step}")
                nc.scalar.activation(c0[:], acc[:], AF.Copy)
                cur = c0
                w = step
                while w > 64:
                    nxt = workp.tile([128, w // 2], F32, tag=f"t{w}")
                    nc.vector.tensor_tensor(
                        out=nxt[:], in0=cur[:, 0 : w // 2], in1=cur[:, w // 2 : w],
                        op=ALU.add,
                    )
                    cur, w = nxt, w // 2
                s2 = workp.tile([128, DIM], F32, tag="s2")
                nc.vector.tensor_tensor(
                    out=s2[:], in0=cur[:], in1=pos[:, pi, :], op=ALU.add
                )
                return s2

            def emit_tp(pi, nm, s2):
                tps = ptr.tile([64, 128], F32, tag="tp")
                nc.tensor.matmul(
                    tps[:], s2[:], eye[:], is_transpose=True, start=True, stop=True,
                )
                s_t = persistp.tile([64, 128], F32, tag=f"sT{nm}{pi}")
                nc.vector.tensor_copy(s_t[:], tps[:])
                sT[(nm, pi)] = s_t

            def emit_R(pi):
                Rps = pR.tile([128, BUCKETS], F32, tag="R")
                for v in range(2):
                    nc.tensor.matmul(
                        Rps[64 * v : 64 * (v + 1), :],
                        sT[("q", pi)][:, 64 * v : 64 * (v + 1)],
                        sT[("k", pi)][:, 64 * v : 64 * (v + 1)],
                        start=True,
                        stop=True,
                    )
                return Rps

            def emit_y(pi, Rps):
                y = workp.tile([128, BUCKETS], F32, tag="y")
                nc.vector.tensor_scalar(
                    out=y[:], in0=Rps[:], scalar1=0.0, scalar2=EPS,
                    op0=ALU.max, op1=ALU.add,
                )
                return y

            def emit_strip(pi, y):
                # K column strip h=pi: exp((ln(relu R + eps))/T) * exp(g/T),
                # with row sums accumulated as the Sinkhorn seed.
                u = workp.tile([128, BUCKETS], F32, tag="u")
                nc.scalar.activation(u[:], y[:], AF.Ln)
                vv = workp.tile([128, BUCKETS], F32, tag="vv")
                nc.scalar.activation(vv[:], u[:], AF.Exp, scale=1.0 / TEMP)
                nc.vector._custom_dve(
                    TENSOR_TENSOR_REDUCE,
                    out=E0b[:, 64 * pi : 64 * (pi + 1)],
                    in0=vv[:],
                    in1=eg[:, pi, :],
                    s0=0.0,
                    s1=1.0,
                    accum_out=seed[:, pi : pi + 1],
                )

            def emit_striptp(pi):
                tstr = pscr.tile([64, 128], BF16, tag="scr")
                nc.tensor.matmul(
                    tstr[:], E0b[:, 64 * pi : 64 * (pi + 1)], eyeh[:],
                    is_transpose=True, start=True, stop=True,
                )
                nc.vector.tensor_copy(KTb[64 * pi : 64 * (pi + 1), :], tstr[:])

            # Emission ordered by data-readiness so no engine queue blocks on
            # a dependency that is satisfied later than its successors' data:
            # chunks complete in issue order (p0q, p0k, p1q, p1k).
            # Cross-engine waits lower to monotone per-engine counters: a
            # consumer waits for "producer engine completed >= N" where N is
            # the producer-queue position at emission time. So every consumer
            # must be emitted IMMEDIATELY after its producer's last
            # instruction on that engine — anything emitted in between
            # becomes a false dependency. Order below is tuned under that
            # rule with chunks completing in issue order.
            acc00 = emit_mms(0, 0)
            s2_00 = emit_drain_tree(acc00, 0, posq)
            acc01 = emit_mms(0, 1)
            s2_01 = emit_drain_tree(acc01, 0, posk)
            emit_tp(0, "q", s2_00)
            acc10 = emit_mms(1, 0)
            emit_tp(0, "k", s2_01)
            s2_10 = emit_drain_tree(acc10, 1, posq)
            # last group with ri=4: a shallower drain tree on the critical tail
            acc11 = emit_mms(1, 1, step=256)
            s2_11 = emit_drain_tree(acc11, 1, posk)
            R0 = emit_R(0)
            emit_tp(1, "q", s2_10)
            y0 = emit_y(0, R0)
            emit_tp(1, "k", s2_11)
            R1 = emit_R(1)
            y1 = emit_y(1, R1)
            emit_strip(0, y0)
            emit_strip(1, y1)

            # ---- Sinkhorn, scale-vector form ----
            # bf16 scale vectors + stationaries: total rel err validated at
            # 5.9e-3 on the host against the 2e-2 budget.
            lp = nc.allow_low_precision(reason="bf16 sinkhorn scale vectors")
            lp.__enter__()
            # a_1 = 1/rowsums: write into M2 block v at cols {v, 2+v}
            for v in range(2):
                sl = slice(64 * v, 64 * (v + 1))
                dst = M2[sl].rearrange("p (h w) -> p h w", h=2)[:, :, v]
                nc.vector.reciprocal(dst, seed[sl, :])
            # KT rows are only needed by the second matvec; emitting them
            # after the seed reciprocals keeps the first matvec unblocked.
            emit_striptp(0)
            emit_striptp(1)

            for t in range(SINKHORN_ITER):
                # b-update: b_raw[(h,j), (h',v')] = sum_i K[(v',i),(h,j)] a[(v',i),h']
                b_raw = pmv.tile([128, 4], F32, tag="mv")
                nc.tensor.matmul(b_raw[:], E0b[:], M2[:], start=True, stop=True)
                # valid cols for block h: {2h, 2h+1} (contiguous)
                nc.vector.reciprocal(M1[0:64, 0:2], b_raw[0:64, 0:2])
                if ACT_RECIP and t > 0:
                    nc.scalar.activation(
                        M1[64:128, 2:4], b_raw[64:128, 2:4], AF.Reciprocal
                    )
                else:
                    nc.vector.reciprocal(M1[64:128, 2:4], b_raw[64:128, 2:4])
                if t == SINKHORN_ITER - 1:
                    break
                # a-update: a_raw[(v,i), (h',v')] = sum_j K[(v,i),(h',j)] b[(h',j),v']
                a_raw = pmv.tile([128, 4], F32, tag="mv")
                nc.tensor.matmul(a_raw[:], KTb[:], M1[:], start=True, stop=True)
                # valid cols for block v: {v, 2+v} (stride 2)
                for v in range(2):
                    sl = slice(64 * v, 64 * (v + 1))
                    src = a_raw[sl].rearrange("p (h w) -> p h w", h=2)[:, :, v]
                    dst = M2[sl].rearrange("p (h w) -> p h w", h=2)[:, :, v]
                    if ACT_RECIP and v == 1 and t > 0:
                        nc.scalar.activation(dst, src, AF.Reciprocal)
                    else:
                        nc.vector.reciprocal(dst, src)
                if t == SINKHORN_ITER - 2:
                    # M2 now holds the final a; fold it into K while the last
                    # b half-step runs: T1[(v,i),(h,j)] = K * a[(v,i),h]
                    for v in range(2):
                        sl = slice(64 * v, 64 * (v + 1))
                        av = M2[sl].rearrange("p (h w) -> p h w", h=2)[:, :, v]
                        nc.vector.tensor_tensor(
                            out=T1[sl].rearrange("p (h j) -> p h j", h=2),
                            in0=E0b[sl].rearrange("p (h j) -> p h j", h=2),
                            in1=av.unsqueeze(-1).broadcast_to((64, 2, BUCKETS)),
                            op=ALU.mult,
                        )

            # ---- assembly: E = T1 * b_replicated ----
            tpb = pscr.tile([64, 128], BF16, tag="scr")
            nc.tensor.matmul(tpb[0:4, :], M1[:], eyeh[:], is_transpose=True, start=True, stop=True)
            bT = smallp.tile([4, 128], BF16, tag="bT")
            nc.vector.tensor_copy(bT[:], tpb[0:4, :])
            # brep[p=(v,i), (h,j)] = b[(h,j), v]: the h'-mask in M1 collapses
            # the selector sum to exactly the matching b value.
            brep = pbrep.tile([128, 2 * BUCKETS], F32, tag="brep")
            nc.tensor.matmul(brep[:], selv4[:], bT[:], start=True, stop=True)
            Efin = persistp.tile([128, 2 * BUCKETS], F32, tag="Efin")
            for h in range(2):
                sl = slice(64 * h, 64 * (h + 1))
                nc.vector.tensor_tensor(
                    out=Efin[:, sl], in0=T1[:, sl], in1=brep[:, sl], op=ALU.mult
                )
            # single output DMA (one SWDGE descriptor-gen pass on gpsimd,
            # which sits idle here; HWDGE pays ~1.4us of issue latency)
            ov = out_d[:].rearrange("(h v) i j -> (v i) h j", h=2, v=2)
            nc.gpsimd.dma_start(ov, Efin[:].rearrange("p (h j) -> p h j", h=2))
            lp.__exit__(None, None, None)

    _preload_act_table(nc)
    nc.compile()
    return nc


# act_info.json act_func_sets index of natural_log_exp_and_others, the one
# table that serves Ln AND Exp (and Relu/Copy). Pre-loading it up front
# makes Bacc's membership-based fixpoint skip every per-activation
# ACT_TABLE_LOAD (1.28us each) for the Exp/Ln phase.
ACT_SET_LN_EXP = 6


def _preload_act_table(nc, set_id=ACT_SET_LN_EXP):
    load = mybir.InstLoadActFuncSet(
        name=nc.get_next_instruction_name(), act_func_set_id=set_id, ins=[], outs=[]
    )
    for blk in nc.main_func.blocks:
        for idx, inst in enumerate(blk.instructions):
            eng = getattr(inst, "engine", None)
            if eng != mybir.EngineType.Activation:
                continue
            if isinstance(
                inst,
                (
                    mybir.InstDMACopy,
                    mybir.InstActivation,
                    mybir.InstLoadActFuncSet,
                ),
            ):
                load.engine = eng
                nc.register_instruction(load)
                blk.instructions.insert(idx, load)
                return
    raise AssertionError("no activation-engine body instruction found")


_NC = None


def _get_program():
    global _NC
    if _NC is None:
        _NC = _build_program()
    return _NC


def _stack_pairs(a):
    # [4, X, T] (bh-major) -> [128=(v,X), pair, T] with bh = 2*pair + v
    x, t = a.shape[1], a.shape[2]
    return np.ascontiguousarray(
        a.reshape(PAIRS, 2, x, t).transpose(1, 2, 0, 3).reshape(2 * x, PAIRS, t)
    )


def _err_diffuse_fp8(x):
    # x: (BH, SEQ, DIM) f32 -> fp8 e4m3 with the quantization residual
    # carried along the 128 seq positions of each bucket, so bucket sums
    # of the quantized values telescope to near-exact.
    nb, _, d = x.shape
    sl = SEQ // BUCKETS
    xb = x.reshape(nb, BUCKETS, sl, d)
    out = np.empty((nb, BUCKETS, sl, d), dtype=ml_dtypes.float8_e4m3fn)
    carry = np.zeros((nb, BUCKETS, d), np.float32)
    for s in range(sl):
        v = xb[:, :, s, :] + carry
        qv = v.astype(ml_dtypes.float8_e4m3fn)
        out[:, :, s, :] = qv
        carry = v - qv.astype(np.float32)
    return out.reshape(nb, SEQ, d)


def _make_in_maps(inputs):
    q8 = _err_diffuse_fp8(np.asarray(inputs["q"], dtype=np.float32))
    k8 = _err_diffuse_fp8(np.asarray(inputs["k"], dtype=np.float32))
    qpe = np.asarray(inputs["q_pos_emb"], dtype=np.float32)
    kpe = np.asarray(inputs["k_pos_emb"], dtype=np.float32)
    g = np.ascontiguousarray(inputs["gumbel"], dtype=np.float32)

    b = BH // HEADS
    qpos = np.broadcast_to(qpe, (b, HEADS, BUCKETS, DIM)).reshape(BH, BUCKETS, DIM)
    kpos = np.broadcast_to(kpe, (b, HEADS, BUCKETS, DIM)).reshape(BH, BUCKETS, DIM)
    eye = np.eye(128, dtype=np.float32)
    eyeh = np.eye(128, dtype=np.float32).astype(ml_dtypes.bfloat16)
    ey = (np.eye(128, dtype=np.float32) / 128.0).astype(ml_dtypes.float8_e4m3fn)
    eyeb2 = np.ascontiguousarray(np.concatenate([ey, ey], axis=1))
    # selv4[c=(2h'+v'), p] = 1 if v' == p//64
    selv4 = np.zeros((4, 128), np.float32)
    for hp in range(2):
        for vp in range(2):
            selv4[2 * hp + vp, 64 * vp : 64 * vp + 64] = 1.0

    in_maps = []
    for c in range(N_CORES):
        sl = slice(NBH * c, NBH * (c + 1))
        in_maps.append(
            {
                "q": np.ascontiguousarray(q8[sl]),
                "k": np.ascontiguousarray(k8[sl]),
                "posq": _stack_pairs(qpos[sl]),
                "posk": _stack_pairs(kpos[sl]),
                "gum": _stack_pairs(g[sl]),
                "eyeb2": eyeb2,
                "eye": eye,
                "eyeh": eyeh,
                "selv4": selv4.astype(ml_dtypes.bfloat16),
            }
        )
    return in_maps


def run(inputs, trace=False):
    nc = _get_program()
    in_maps = _make_in_maps(inputs)
    res = run_bass_kernel_spmd(
        nc, in_maps, core_ids=list(range(N_CORES)), trace=trace
    )
    out = np.concatenate(
        [res.results[c]["out"] for c in range(N_CORES)], axis=0
    ).astype(np.float32)
    return out, res


def kernel(**inputs) -> np.ndarray:
    out, _ = run(inputs, trace=False)
    return out


# revision 60
# speedup vs baseline: 1.0898x; 1.0163x over previous
"""Trainium2 Bass kernel for nn_AttentionSortNet (sparse_attention).

Per bh slice (data-parallel over bh across 8 cores):
  b_q = bucket-mean(q), b_k = bucket-mean(k)          (64 buckets x 128 elems)
  sq = b_q + q_pos, sk = b_k + k_pos
  R  = sq @ sk^T                                       (64 x 64)
  K  = exp((ln(relu(R)+eps) + gumbel) / T)
  8x Sinkhorn; out = normalized K

Design (per core, 4 bh = 2 bh-pairs; ~46.5us vs 58.5us bf16 baseline):
  - q/k staged as fp8 e4m3 with per-bucket error diffusion on the host:
    the quantization residual is carried along the 128 seq positions of
    each bucket, so bucket SUMS of the fp8 values are near-exact
    (4.5e-3 total rel err vs the 2e-2 budget). Halves HBM traffic vs
    bf16 to ~4.2 MiB/core (~13 us stream).
  - bucket means entirely on the PE as fp8 DoubleRow matmuls against a
    duplicated scaled eye ([128, 2, 128] stationary): each instruction
    consumes 1024 elems/partition (2 elem/cycle/partition at full
    clock), accumulating ri-lane partial means in PSUM. The ACT engine
    drains PSUM (Copy, in every table), a contiguous halving tree on
    DVE sums the lanes, pos_emb rides the last add. The last group uses
    ri=4 for a shallower tail tree.
  - the tensor engine needs ~3us of CONTINUOUS execution to reach full
    clock (427->216ns per 512-row mm) and any ~0.4us gap drops it
    back: NWARM dummy matmuls warm it up under the DMA stream, and the
    emission order keeps the PE queue dense.
  - cross-engine waits lower to monotone per-engine counters (consumer
    waits "producer engine completed >= N", N = producer-queue position
    at emission). Consumers are emitted immediately after their
    producers; unrelated fill work comes after, never between.
  - chunk completion follows descriptor-generation order, but rows
    below ~2KB make descriptors that clog single DMA engines (the
    stream tail dribbles): 4KB rows, with 2KB rows only for the last
    group's tail chunks.
  - Sinkhorn in scale-vector form: E = diag(a) K diag(b) with
    a <- 1/(K b), b <- 1/(K^T a) -- algebraically identical to the
    reference iteration. Each half-step is ONE 4-column matvec (bf16
    stationary K or K^T, masked bf16 moving columns per (v,h') bh)
    plus two [64,2] DVE reciprocals (~0.65us/half-step), instead of
    rescaling the full 128x128 matrix. The mask layout makes unused
    moving columns stay zero across iterations (memset once).
  - final assembly E = (K*a_bcast) * b_replicated: a is applied per
    partition block during the last matvec's window; b is replicated
    across partitions with one small selector matmul (selv4 [4,128],
    the h'-mask collapses the selector sum), one output DMA on the
    idle gpsimd SWDGE queue (HWDGE pays ~1.4us issue latency there).
  - one ACT table set (natural_log_exp_and_others) force-loaded up
    front via a hand-inserted InstLoadActFuncSet (no per-activation
    table-switch thrash for eg/Ln/Exp).
"""
import sys

sys.path.insert(0, "/opt/trn_rl_repo")

import numpy as np
import ml_dtypes

import concourse.bass as bass
import concourse.bacc as bacc
import concourse.mybir as mybir
from concourse import tile
from concourse.bass_utils import run_bass_kernel_spmd
from concourse.dve_ops import TENSOR_TENSOR_REDUCE

HEADS = 8
BUCKETS = 64
DIM = 64
TEMP = 0.7
EPS = 1e-6
N_CORES = 8
BH = 32
SEQ = 8192
NBH = BH // N_CORES        # 4 bh per core
PAIRS = NBH // 2           # 2 bh-pairs per core
SINKHORN_ITER = 8

F32 = mybir.dt.float32
BF16 = mybir.dt.bfloat16
FP8 = mybir.dt.float8e4
AF = mybir.ActivationFunctionType
AX = mybir.AxisListType
ALU = mybir.AluOpType
DR = mybir.MatmulPerfMode.DoubleRow

# per (pair, tensor): seq elems per partition = 8192 (fp8 = 8KB rows).
# Uniform 4KB rows: small-row chunks generate tiny DMA descriptors that
# pile up on one engine and make the stream tail dribble for ~3us.
CHUNK_SZ = {
    (0, 0): (4096, 4096),
    (0, 1): (4096, 4096),
    (1, 0): (4096, 4096),
    (1, 1): (4096, 2048, 2048),
}

# ACT Reciprocal is rejected by bass (known accuracy issues), so both
# per-half-step reciprocals run sequentially on DVE.
ACT_RECIP = False


def _build_program():
    nc = bacc.Bacc("TRN2", target_bir_lowering=False, debug=False, num_devices=N_CORES)

    q_d = nc.dram_tensor("q", [NBH, SEQ, DIM], FP8, kind="ExternalInput")
    k_d = nc.dram_tensor("k", [NBH, SEQ, DIM], FP8, kind="ExternalInput")
    # pre-stacked on host: [128=(v,row), pair, 64]
    qp_d = nc.dram_tensor("posq", [128, PAIRS, DIM], F32, kind="ExternalInput")
    kp_d = nc.dram_tensor("posk", [128, PAIRS, DIM], F32, kind="ExternalInput")
    g_d = nc.dram_tensor("gum", [128, PAIRS, BUCKETS], F32, kind="ExternalInput")
    eyeb2_d = nc.dram_tensor("eyeb2", [128, 256], FP8, kind="ExternalInput")
    eye_d = nc.dram_tensor("eye", [128, 128], F32, kind="ExternalInput")
    eyeh_d = nc.dram_tensor("eyeh", [128, 128], BF16, kind="ExternalInput")
    sel_d = nc.dram_tensor("selv4", [4, 128], BF16, kind="ExternalInput")
    out_d = nc.dram_tensor("out", [NBH, BUCKETS, BUCKETS], F32, kind="ExternalOutput")

    with tile.TileContext(nc) as tc:
        with (
            tc.tile_pool(name="const", bufs=1) as constp,
            tc.tile_pool(name="data", bufs=13) as datap,
            tc.tile_pool(name="work", bufs=3) as workp,
            tc.tile_pool(name="small", bufs=4) as smallp,
            tc.tile_pool(name="persist", bufs=1) as persistp,
            tc.tile_pool(name="pacc", bufs=2, space=bass.MemorySpace.PSUM) as pacc,
            tc.tile_pool(name="ptr", bufs=2, space=bass.MemorySpace.PSUM) as ptr,
            tc.tile_pool(name="pR", bufs=1, space=bass.MemorySpace.PSUM) as pR,
            tc.tile_pool(name="pmv", bufs=1, space=bass.MemorySpace.PSUM) as pmv,
            tc.tile_pool(name="pscr", bufs=1, space=bass.MemorySpace.PSUM) as pscr,
            tc.tile_pool(name="pbrep", bufs=1, space=bass.MemorySpace.PSUM) as pbrep,
        ):
            # small consts FIRST on the sync HWDGE queue: their descriptors
            # must hit the DMA engines before the ~4 MiB of q/k descriptors,
            # or eyeb2/gum only land after the whole stream drains.
            eyeb2 = constp.tile([128, 256], FP8, tag="eyeb2")
            nc.sync.dma_start(eyeb2[:], eyeb2_d[:])
            gum = constp.tile([128, PAIRS, BUCKETS], F32, tag="gum")
            nc.sync.dma_start(gum[:], g_d[:])
            posq = constp.tile([128, PAIRS, DIM], F32, tag="posq")
            nc.sync.dma_start(posq[:], qp_d[:])
            posk = constp.tile([128, PAIRS, DIM], F32, tag="posk")
            nc.sync.dma_start(posk[:], kp_d[:])
            eye = constp.tile([128, 128], F32, tag="eye")
            nc.sync.dma_start(eye[:], eye_d[:])
            eyeh = constp.tile([128, 128], BF16, tag="eyeh")
            nc.sync.dma_start(eyeh[:], eyeh_d[:])
            selv4 = constp.tile([4, 128], BF16, tag="selv4")
            nc.sync.dma_start(selv4[:], sel_d[:])

            # big q/k chunks ride the gpsimd SWDGE queue
            chunk_map = {}
            for pi in range(PAIRS):
                for ti, src in ((0, q_d), (1, k_d)):
                    view = src[2 * pi : 2 * pi + 2].rearrange(
                        "b (bu sl) d -> (b bu) (sl d)", bu=BUCKETS, sl=SEQ // BUCKETS
                    )
                    off = 0
                    lst = []
                    for csz in CHUNK_SZ[(pi, ti)]:
                        ch = datap.tile([128, csz], FP8, tag="data")
                        nc.gpsimd.dma_start(ch[:], view[:, off : off + csz])
                        lst.append((ch, off, csz))
                        off += csz
                    chunk_map[(pi, ti)] = lst

            # eg = exp(g/T) during the DMA window (ACT is otherwise idle)
            eg = constp.tile([128, PAIRS, BUCKETS], F32, tag="eg")
            nc.scalar.activation(eg[:], gum[:], AF.Exp, scale=1.0 / TEMP)
            # Ln table warm during the DMA window
            tw = constp.tile([128, 1], F32, tag="tw")
            nc.vector.memset(tw[:], 1.0)
            nc.scalar.activation(tw[:], tw[:], AF.Ln)

            # Sinkhorn state: masked moving tiles (bf16: the matvec operands
            # and stationaries are bf16, validated 5.9e-3 total rel err).
            # Columns are (h',v') with c = 2h'+v'. Unwritten positions must
            # stay zero -> memset once.
            M1 = persistp.tile([128, 4], BF16, tag="M1")   # b side, part (h,j)
            M2 = persistp.tile([128, 4], BF16, tag="M2")   # a side, part (v,i)
            nc.vector.memset(M1[:], 0.0)
            nc.vector.memset(M2[:], 0.0)
            seed = persistp.tile([128, PAIRS], F32, tag="seed")
            E0b = persistp.tile([128, 2 * BUCKETS], BF16, tag="E0b")  # K (bf16 everywhere)
            KTb = persistp.tile([128, 2 * BUCKETS], BF16, tag="KTb")  # K^T bf16 stationary
            T1 = persistp.tile([128, 2 * BUCKETS], F32, tag="T1")    # K*a

            eyeb2v = eyeb2[:].rearrange("p (two m) -> p two m", two=2)
            sT = {}

            # PE p-state warmup: the tensor engine needs ~3us of continuous
            # execution to reach full clock (427ns -> 216ns per 512-row mm).
            # Run dummy DoubleRow matmuls on scratch data while the first
            # chunk streams in, so the real matmuls start at full speed.
            wsrc = constp.tile([128, 1024], FP8, tag="wsrc")
            nc.vector.memset(wsrc[:], 0.0)
            wacc = pacc.tile([128, 512], F32, tag="acc")
            NWARM = 8
            for i in range(NWARM):
                nc.tensor.matmul(
                    wacc[:],
                    eyeb2v,
                    wsrc[:].rearrange("p (two f) -> p two f", two=2),
                    start=(i == 0),
                    stop=(i == NWARM - 1),
                    perf_mode=DR,
                )

            def emit_mms(pi, ti, step=512):
                # DoubleRow eye-pair matmuls: each consumes 2*step elems per
                # partition, accumulating step "ri-lane" partial means.
                acc = pacc.tile([128, step], F32, tag="acc")
                total = SEQ // (2 * step)
                m = 0
                for ch, coff, csz in chunk_map[(pi, ti)]:
                    for l in range(0, csz, 2 * step):
                        nc.tensor.matmul(
                            acc[:],
                            eyeb2v,
                            ch[:, l : l + 2 * step].rearrange(
                                "p (two f) -> p two f", two=2
                            ),
                            start=(m == 0),
                            stop=(m == total - 1),
                            perf_mode=DR,
                        )
                        m += 1
                return acc, step

            def emit_drain_tree(accs, pi, pos):
                # halving tree over the ri lanes, all on DVE: dual-PSUM reads
                # are forbidden, but copy-half + one-PSUM-input add does the
                # first level in one engine queue (no ACT drain hop). The
                # pos_emb add is the last level.
                acc, step = accs
                half = step // 2
                ca = workp.tile([128, half], F32, tag=f"ca{step}")
                nc.vector.tensor_copy(ca[:], acc[:, 0:half])
                cur = workp.tile([128, half], F32, tag=f"t{step}")
                nc.vector.tensor_tensor(
                    out=cur[:], in0=acc[:, half:step], in1=ca[:], op=ALU.add
                )
                w = half
                while w > 64:
                    nxt = workp.tile([128, w // 2], F32, tag=f"t{w}")
                    nc.vector.tensor_tensor(
                        out=nxt[:], in0=cur[:, 0 : w // 2], in1=cur[:, w // 2 : w],
                        op=ALU.add,
                    )
                    cur, w = nxt, w // 2
                s2 = workp.tile([128, DIM], F32, tag="s2")
                nc.vector.tensor_tensor(
                    out=s2[:], in0=cur[:], in1=pos[:, pi, :], op=ALU.add
                )
                return s2

            def emit_tp(pi, nm, s2):
                tps = ptr.tile([64, 128], F32, tag="tp")
                nc.tensor.matmul(
                    tps[:], s2[:], eye[:], is_transpose=True, start=True, stop=True,
                )
                s_t = persistp.tile([64, 128], F32, tag=f"sT{nm}{pi}")
                nc.vector.tensor_copy(s_t[:], tps[:])
                sT[(nm, pi)] = s_t

            def emit_R(pi):
                Rps = pR.tile([128, BUCKETS], F32, tag="R")
                for v in range(2):
                    nc.tensor.matmul(
                        Rps[64 * v : 64 * (v + 1), :],
                        sT[("q", pi)][:, 64 * v : 64 * (v + 1)],
                        sT[("k", pi)][:, 64 * v : 64 * (v + 1)],
                        start=True,
                        stop=True,
                    )
                return Rps

            def emit_y(pi, Rps):
                y = workp.tile([128, BUCKETS], F32, tag="y")
                nc.vector.tensor_scalar(
                    out=y[:], in0=Rps[:], scalar1=0.0, scalar2=EPS,
                    op0=ALU.max, op1=ALU.add,
                )
                return y

            def emit_strip(pi, y):
                # K column strip h=pi: exp((ln(relu R + eps))/T) * exp(g/T),
                # with row sums accumulated as the Sinkhorn seed.
                u = workp.tile([128, BUCKETS], F32, tag="u")
                nc.scalar.activation(u[:], y[:], AF.Ln)
                vv = workp.tile([128, BUCKETS], F32, tag="vv")
                nc.scalar.activation(vv[:], u[:], AF.Exp, scale=1.0 / TEMP)
                nc.vector._custom_dve(
                    TENSOR_TENSOR_REDUCE,
                    out=E0b[:, 64 * pi : 64 * (pi + 1)],
                    in0=vv[:],
                    in1=eg[:, pi, :],
                    s0=0.0,
                    s1=1.0,
                    accum_out=seed[:, pi : pi + 1],
                )

            def emit_striptp(pi):
                tstr = pscr.tile([64, 128], BF16, tag="scr")
                nc.tensor.matmul(
                    tstr[:], E0b[:, 64 * pi : 64 * (pi + 1)], eyeh[:],
                    is_transpose=True, start=True, stop=True,
                )
                nc.vector.tensor_copy(KTb[64 * pi : 64 * (pi + 1), :], tstr[:])

            # Emission ordered by data-readiness so no engine queue blocks on
            # a dependency that is satisfied later than its successors' data:
            # chunks complete in issue order (p0q, p0k, p1q, p1k).
            # Cross-engine waits lower to monotone per-engine counters: a
            # consumer waits for "producer engine completed >= N" where N is
            # the producer-queue position at emission time. So every consumer
            # must be emitted IMMEDIATELY after its producer's last
            # instruction on that engine — anything emitted in between
            # becomes a false dependency. Order below is tuned under that
            # rule with chunks completing in issue order.
            acc00 = emit_mms(0, 0)
            s2_00 = emit_drain_tree(acc00, 0, posq)
            acc01 = emit_mms(0, 1)
            s2_01 = emit_drain_tree(acc01, 0, posk)
            emit_tp(0, "q", s2_00)
            acc10 = emit_mms(1, 0)
            emit_tp(0, "k", s2_01)
            s2_10 = emit_drain_tree(acc10, 1, posq)
            # last group with ri=4: a shallower drain tree on the critical tail
            acc11 = emit_mms(1, 1, step=256)
            s2_11 = emit_drain_tree(acc11, 1, posk)
            R0 = emit_R(0)
            emit_tp(1, "q", s2_10)
            y0 = emit_y(0, R0)
            emit_tp(1, "k", s2_11)
            R1 = emit_R(1)
            y1 = emit_y(1, R1)
            emit_strip(0, y0)
            emit_strip(1, y1)

            # ---- Sinkhorn, scale-vector form ----
            # bf16 scale vectors + stationaries: total rel err validated at
            # 5.9e-3 on the host against the 2e-2 budget.
            lp = nc.allow_low_precision(reason="bf16 sinkhorn scale vectors")
            lp.__enter__()
            # a_1 = 1/rowsums: write into M2 block v at cols {v, 2+v}
            for v in range(2):
                sl = slice(64 * v, 64 * (v + 1))
                dst = M2[sl].rearrange("p (h w) -> p h w", h=2)[:, :, v]
                nc.vector.reciprocal(dst, seed[sl, :])
            # KT rows are only needed by the second matvec; emitting them
            # after the seed reciprocals keeps the first matvec unblocked.
            emit_striptp(0)
            emit_striptp(1)

            for t in range(SINKHORN_ITER):
                # b-update: b_raw[(h,j), (h',v')] = sum_i K[(v',i),(h,j)] a[(v',i),h']
                b_raw = pmv.tile([128, 4], F32, tag="mv")
                nc.tensor.matmul(b_raw[:], E0b[:], M2[:], start=True, stop=True)
                # valid cols for block h: {2h, 2h+1} (contiguous)
                nc.vector.reciprocal(M1[0:64, 0:2], b_raw[0:64, 0:2])
                if ACT_RECIP and t > 0:
                    nc.scalar.activation(
                        M1[64:128, 2:4], b_raw[64:128, 2:4], AF.Reciprocal
                    )
                else:
                    nc.vector.reciprocal(M1[64:128, 2:4], b_raw[64:128, 2:4])
                if t == SINKHORN_ITER - 1:
                    break
                # a-update: a_raw[(v,i), (h',v')] = sum_j K[(v,i),(h',j)] b[(h',j),v']
                a_raw = pmv.tile([128, 4], F32, tag="mv")
                nc.tensor.matmul(a_raw[:], KTb[:], M1[:], start=True, stop=True)
                # valid cols for block v: {v, 2+v} (stride 2)
                for v in range(2):
                    sl = slice(64 * v, 64 * (v + 1))
                    src = a_raw[sl].rearrange("p (h w) -> p h w", h=2)[:, :, v]
                    dst = M2[sl].rearrange("p (h w) -> p h w", h=2)[:, :, v]
                    if ACT_RECIP and v == 1 and t > 0:
                        nc.scalar.activation(dst, src, AF.Reciprocal)
                    else:
                        nc.vector.reciprocal(dst, src)
                if t == SINKHORN_ITER - 2:
                    # M2 now holds the final a; fold it into K while the last
                    # b half-step runs: T1[(v,i),(h,j)] = K * a[(v,i),h]
                    for v in range(2):
                        sl = slice(64 * v, 64 * (v + 1))
                        av = M2[sl].rearrange("p (h w) -> p h w", h=2)[:, :, v]
                        nc.vector.tensor_tensor(
                            out=T1[sl].rearrange("p (h j) -> p h j", h=2),
                            in0=E0b[sl].rearrange("p (h j) -> p h j", h=2),
                            in1=av.unsqueeze(-1).broadcast_to((64, 2, BUCKETS)),
                            op=ALU.mult,
                        )

            # ---- assembly: E = T1 * b_replicated ----
            tpb = pscr.tile([64, 128], BF16, tag="scr")
            nc.tensor.matmul(tpb[0:4, :], M1[:], eyeh[:], is_transpose=True, start=True, stop=True)
            bT = smallp.tile([4, 128], BF16, tag="bT")
            nc.vector.tensor_copy(bT[:], tpb[0:4, :])
            # brep[p=(v,i), (h,j)] = b[(h,j), v]: the h'-mask in M1 collapses
            # the selector sum to exactly the matching b value.
            brep = pbrep.tile([128, 2 * BUCKETS], F32, tag="brep")
            nc.tensor.matmul(brep[:], selv4[:], bT[:], start=True, stop=True)
            Efin = persistp.tile([128, 2 * BUCKETS], F32, tag="Efin")
            for h in range(2):
                sl = slice(64 * h, 64 * (h + 1))
                nc.vector.tensor_tensor(
                    out=Efin[:, sl], in0=T1[:, sl], in1=brep[:, sl], op=ALU.mult
                )
            # single output DMA (one SWDGE descriptor-gen pass on gpsimd,
            # which sits idle here; HWDGE pays ~1.4us of issue latency)
            ov = out_d[:].rearrange("(h v) i j -> (v i) h j", h=2, v=2)
            nc.gpsimd.dma_start(ov, Efin[:].rearrange("p (h j) -> p h j", h=2))
            lp.__exit__(None, None, None)

    _preload_act_table(nc)
    nc.compile()
    return nc


# act_info.json act_func_sets index of natural_log_exp_and_others, the one
# table that serves Ln AND Exp (and Relu/Copy). Pre-loading it up front
# makes Bacc's membership-based fixpoint skip every per-activation
# ACT_TABLE_LOAD (1.28us each) for the Exp/Ln phase.
ACT_SET_LN_EXP = 6


def _preload_act_table(nc, set_id=ACT_SET_LN_EXP):
    load = mybir.InstLoadActFuncSet(
        name=nc.get_next_instruction_name(), act_func_set_id=set_id, ins=[], outs=[]
    )
    for blk in nc.main_func.blocks:
        for idx, inst in enumerate(blk.instructions):
            eng = getattr(inst, "engine", None)
            if eng != mybir.EngineType.Activation:
                continue
            if isinstance(
                inst,
                (
                    mybir.InstDMACopy,
                    mybir.InstActivation,
                    mybir.InstLoadActFuncSet,
                ),
            ):
                load.engine = eng
                nc.register_instruction(load)
                blk.instructions.insert(idx, load)
                return
    raise AssertionError("no activation-engine body instruction found")


_NC = None


def _get_program():
    global _NC
    if _NC is None:
        _NC = _build_program()
    return _NC


def _stack_pairs(a):
    # [4, X, T] (bh-major) -> [128=(v,X), pair, T] with bh = 2*pair + v
    x, t = a.shape[1], a.shape[2]
    return np.ascontiguousarray(
        a.reshape(PAIRS, 2, x, t).transpose(1, 2, 0, 3).reshape(2 * x, PAIRS, t)
    )


def _err_diffuse_fp8(x):
    # x: (BH, SEQ, DIM) f32 -> fp8 e4m3 with the quantization residual
    # carried along the 128 seq positions of each bucket, so bucket sums
    # of the quantized values telescope to near-exact.
    nb, _, d = x.shape
    sl = SEQ // BUCKETS
    xb = x.reshape(nb, BUCKETS, sl, d)
    out = np.empty((nb, BUCKETS, sl, d), dtype=ml_dtypes.float8_e4m3fn)
    carry = np.zeros((nb, BUCKETS, d), np.float32)
    for s in range(sl):
        v = xb[:, :, s, :] + carry
        qv = v.astype(ml_dtypes.float8_e4m3fn)
        out[:, :, s, :] = qv
        carry = v - qv.astype(np.float32)
    return out.reshape(nb, SEQ, d)


def _make_in_maps(inputs):
    q8 = _err_diffuse_fp8(np.asarray(inputs["q"], dtype=np.float32))
    k8 = _err_diffuse_fp8(np.asarray(inputs["k"], dtype=np.float32))
    qpe = np.asarray(inputs["q_pos_emb"], dtype=np.float32)
    kpe = np.asarray(inputs["k_pos_emb"], dtype=np.float32)
    g = np.ascontiguousarray(inputs["gumbel"], dtype=np.float32)

    b = BH // HEADS
    qpos = np.broadcast_to(qpe, (b, HEADS, BUCKETS, DIM)).reshape(BH, BUCKETS, DIM)
    kpos = np.broadcast_to(kpe, (b, HEADS, BUCKETS, DIM)).reshape(BH, BUCKETS, DIM)
    eye = np.eye(128, dtype=np.float32)
    eyeh = np.eye(128, dtype=np.float32).astype(ml_dtypes.bfloat16)
    ey = (np.eye(128, dtype=np.float32) / 128.0).astype(ml_dtypes.float8_e4m3fn)
    eyeb2 = np.ascontiguousarray(np.concatenate([ey, ey], axis=1))
    # selv4[c=(2h'+v'), p] = 1 if v' == p//64
    selv4 = np.zeros((4, 128), np.float32)
    for hp in range(2):
        for vp in range(2):
            selv4[2 * hp + vp, 64 * vp : 64 * vp + 64] = 1.0

    in_maps = []
    for c in range(N_CORES):
        sl = slice(NBH * c, NBH * (c + 1))
        in_maps.append(
            {
                "q": np.ascontiguousarray(q8[sl]),
                "k": np.ascontiguousarray(k8[sl]),
                "posq": _stack_pairs(qpos[sl]),
                "posk": _stack_pairs(kpos[sl]),
                "gum": _stack_pairs(g[sl]),
                "eyeb2": eyeb2,
                "eye": eye,
                "eyeh": eyeh,
                "selv4": selv4.astype(ml_dtypes.bfloat16),
            }
        )
    return in_maps


def run(inputs, trace=False):
    nc = _get_program()
    in_maps = _make_in_maps(inputs)
    res = run_bass_kernel_spmd(
        nc, in_maps, core_ids=list(range(N_CORES)), trace=trace
    )
    out = np.concatenate(
        [res.results[c]["out"] for c in range(N_CORES)], axis=0
    ).astype(np.float32)
    return out, res


def kernel(**inputs) -> np.ndarray:
    out, _ = run(inputs, trace=False)
    return out
